# revision 16
# baseline (speedup 1.0000x reference)
"""Trainium2 Bass kernel for nn_Device_Policy (segment_reduce).

Strategy: shard the node axis N across 8 NeuronCores.  Host marshals one
interleaved bf16 tensor X[N, 256] per row:
    [ maskT(64) | mpnn(128) | state(64) ]
(mask 0/1 is exact in bf16; mpnn/state bf16 rounding ~0.4% rel, well under
the 2e-2 gate).  Rows are self-contained, so nodes can be laid out in any
(partition, slot) order: each SBUF tile is loaded as one fully-contiguous
16 KiB run per partition (128 descriptors per 2 MiB tile -> DMA runs at
full HBM bandwidth).  Per 128-node chunk the PE does one bf16 matmul
    lhsT = maskT  (K=128 nodes, M=64)   rhs = mpnn  (N=128)
accumulating dse[d, h] into a single PSUM [64, 128] over all 256 chunks.
State column sums / sums-of-squares accumulate on the otherwise-idle DVE
(f32 accumulators, scalar-engine Square), finished with two tiny
ones-matmuls.  The [64, 130] partial is AllReduce'd across the 8 cores and
the small replicated MLP head runs after (device-feat embedding
precomputed before the loop).
"""

import sys

if "/opt/trn_rl_repo" not in sys.path:
    sys.path.insert(0, "/opt/trn_rl_repo")

import ml_dtypes
import numpy as np

import concourse.bacc as bacc
import concourse.bass as bass
import concourse.mybir as mybir
import concourse.tile as tile
from concourse import masks
from concourse.bass_utils import run_bass_kernel_spmd

NCORES = 8
N = 262144
F = 64
D = 64
DF = 32
H1 = 128
H2 = 64
NSH = N // NCORES          # nodes per core = 32768
TN = 4096                  # nodes per tile
NT = NSH // TN             # 8 tiles per core
R = TN // 128              # 32 row-slots per partition = chunks per tile
XW = 256                   # shipped row width (bf16): mask|mpnn|state
EPS = 1e-6
SLOPE = 0.1
# Fused Lrelu mis-computes on HW (alpha is not honored) and CoreSim does not
# implement it either -- keep the mul+max decomposition.
USE_LRELU = False

f32 = mybir.dt.float32
bf16 = mybir.dt.bfloat16
fp8 = mybir.dt.float8e4
u16 = mybir.dt.uint16
ADD = mybir.AluOpType.add
MUL = mybir.AluOpType.mult
SUB = mybir.AluOpType.subtract
AX = mybir.AxisListType.X
LRELU = mybir.ActivationFunctionType.Lrelu
IDENT = mybir.ActivationFunctionType.Identity
SQUARE = mybir.ActivationFunctionType.Square
SQRT = mybir.ActivationFunctionType.Sqrt


def build_program(stage="full"):
    nc = bacc.Bacc(
        "TRN2",
        target_bir_lowering=False,
        debug=False,
        enable_asserts=False,
        num_devices=NCORES,
    )

    # one interleaved row per node, shipped as raw u16:
    # [ mask fp8 (32 u16) | mpnn bf16 (128 u16) | state fp8 (32 u16) ]
    x_raw = nc.dram_tensor("x_raw", [NSH, 192], u16, kind="ExternalInput")
    x_dfsT = nc.dram_tensor("x_dfsT", [64, D], f32, kind="ExternalInput")
    x_w1T = nc.dram_tensor("x_w1T", [64, H1], f32, kind="ExternalInput")
    x_b1 = nc.dram_tensor("x_b1", [H1, 1], f32, kind="ExternalInput")
    x_w2T = nc.dram_tensor("x_w2T", [F, H1], f32, kind="ExternalInput")
    x_b2 = nc.dram_tensor("x_b2", [H1, 1], f32, kind="ExternalInput")
    x_w3Tp = nc.dram_tensor("x_w3Tp", [H1, 4 * H2], f32, kind="ExternalInput")
    x_b3 = nc.dram_tensor("x_b3", [H2, 1], f32, kind="ExternalInput")
    x_w4T = nc.dram_tensor("x_w4T", [H2, 1], f32, kind="ExternalInput")
    x_b4 = nc.dram_tensor("x_b4", [D, 1], f32, kind="ExternalInput")
    x_spred = nc.dram_tensor("x_spred", [F, 1], f32, kind="ExternalInput")
    x_mpred = nc.dram_tensor("x_mpred", [H1, 1], f32, kind="ExternalInput")
    y_out = nc.dram_tensor("y_out", [D], f32, kind="ExternalOutput")
    y_dbg = None
    if stage != "full":
        y_dbg = nc.dram_tensor("y_dbg", [64, 130], f32, kind="ExternalOutput")

    with tile.TileContext(nc) as tc:
        emit(nc, tc, x_raw, x_dfsT, x_w1T, x_b1, x_w2T, x_b2, x_w3Tp, x_b3,
             x_w4T, x_b4, x_spred, x_mpred, y_out, stage=stage, y_dbg=y_dbg)

    nc.compile()
    return nc


def emit(nc, tc, x_raw, x_dfsT, x_w1T, x_b1, x_w2T, x_b2, x_w3Tp, x_b3,
         x_w4T, x_b4, x_spred, x_mpred, y_out, stage="full", y_dbg=None):
    ctx_pools = []

    def pool(name, bufs, space="SBUF"):
        p = tc.tile_pool(name=name, bufs=bufs, space=space)
        ctx_pools.append(p)
        return p.__enter__()

    cpool = pool("const", 1)
    big_pool = pool("big", 4)
    sq_pool = pool("sq", 4)
    ep_pool = pool("ep", 1)
    dsepsum_pool = pool("dsepsum", 1, space="PSUM")
    eppsum_pool = pool("eppsum", 2, space="PSUM")
    dram_pool = pool("dram", 1, space="DRAM")

    def leaky(out_t, psum_in, bias):
        if USE_LRELU:
            nc.scalar.activation(out_t, psum_in, LRELU, bias=bias, alpha=SLOPE)
        else:
            nc.scalar.activation(out_t, psum_in, IDENT, bias=bias)
            sc = ep_pool.tile(list(out_t.shape), f32, name="lk", tag="lk")
            nc.vector.tensor_scalar_mul(sc[:, :], out_t, SLOPE)
            nc.vector.tensor_max(out_t, out_t, sc[:, :])

    # ---- constants ----
    ident = cpool.tile([128, 128], f32, name="ident")
    masks.make_identity(nc, ident[:, :])
    zeros = cpool.tile([128, D], f32, name="zeros")
    nc.vector.memset(zeros[:, :], 0.0)

    # ---- warm up the CC engine with a tiny dummy collective: the first
    # collective trigger pays ~10us of CC launch; doing it early (overlapped
    # with the loop) makes the real AllReduce start promptly ----
    dumb_in = dram_pool.tile([1, 1], f32, name="dumb_in", tag="dumb_in")
    dumb_out = dram_pool.tile([1, 1], f32, name="dumb_out", tag="dumb_out",
                              addr_space="Shared")
    nc.scalar.dma_start(dumb_in[:, :], zeros[0:1, 0:1])
    nc.gpsimd.collective_compute(
        "AllReduce",
        ADD,
        replica_groups=[list(range(NCORES))],
        ins=[dumb_in[:, :].opt()],
        outs=[dumb_out[:, :].opt()],
    )

    ones_bf = cpool.tile([128, 1], bf16, name="ones_bf")
    nc.vector.memset(ones_bf[:, :], 1.0)
    ones_f8 = cpool.tile([128, 1], fp8, name="ones_f8")
    nc.vector.memset(ones_f8[:, :], 1.0)

    dfsT = cpool.tile([64, D], f32, name="dfsT")
    nc.scalar.dma_start(dfsT[:, :], x_dfsT[:, :])
    w1T = cpool.tile([64, H1], f32, name="w1T")
    nc.scalar.dma_start(w1T[:, :], x_w1T[:, :])
    b1 = cpool.tile([H1, 1], f32, name="b1")
    nc.scalar.dma_start(b1[:, :], x_b1[:, :])
    w2T = cpool.tile([F, H1], f32, name="w2T")
    nc.scalar.dma_start(w2T[:, :], x_w2T[:, :])
    b2 = cpool.tile([H1, 1], f32, name="b2")
    nc.scalar.dma_start(b2[:, :], x_b2[:, :])
    w3Tp = cpool.tile([H1, 4 * H2], f32, name="w3Tp")
    nc.scalar.dma_start(w3Tp[:, :], x_w3Tp[:, :])
    b3 = cpool.tile([H2, 1], f32, name="b3")
    nc.scalar.dma_start(b3[:, :], x_b3[:, :])
    w4T = cpool.tile([H2, 1], f32, name="w4T")
    nc.scalar.dma_start(w4T[:, :], x_w4T[:, :])
    b4 = cpool.tile([D, 1], f32, name="b4")
    nc.scalar.dma_start(b4[:, :], x_b4[:, :])
    spred = cpool.tile([F, 1], f32, name="spred")
    nc.scalar.dma_start(spred[:, :], x_spred[:, :])
    mpred = cpool.tile([H1, 1], f32, name="mpred")
    nc.scalar.dma_start(mpred[:, :], x_mpred[:, :])

    # ---- main loop ----
    # small first tiles (earlier first matmul) and small last tiles
    # (shorter drain); 384 B per node row across three tensors
    tiles = []
    off = 0
    for n in [2048, 2048, 4096, 4096, 4096, 4096, 4096, 4096, 2048, 2048]:
        tiles.append((off, n))
        off += n
    assert off == NSH
    last_t = len(tiles) - 1
    psum_dse = dsepsum_pool.tile([D, H1], f32, name="psum_dse", tag="psum_dse")
    psum_ssum = dsepsum_pool.tile([1, 512], f32, name="psum_ssum", tag="pss")
    psum_ssq = dsepsum_pool.tile([1, 512], f32, name="psum_ssq", tag="psq")
    for t, (off, n) in enumerate(tiles):
        rr = n // 128
        kk = rr // 8
        big = big_pool.tile([128, R * 192], u16, name="big", tag="big")
        bigv = big[:, :].rearrange("p (r f) -> p r f", f=192)
        # node = off + rr*p + r: each partition is one contiguous 12 KiB run
        nc.sync.dma_start(
            big[:, 0:rr * 192],
            x_raw[off:off + n, :].rearrange("(p r) f -> p (r f)", r=rr))
        sq = sq_pool.tile([128, R * 64], bf16, name="sq", tag="sq")
        for r in range(rr):
            nc.tensor.matmul(
                psum_dse[:, :],
                lhsT=bigv[:, r, 0:32].bitcast(fp8),
                rhs=bigv[:, r, 32:160].bitcast(bf16),
                start=(t == 0 and r == 0),
                stop=(t == last_t and r == rr - 1),
            )
        # state column sums / sums-of-squares ride the PE too: ones-weight
        # matmuls reduce over the partition (node) axis, 512 columns a pop
        for k in range(kk):
            ksl = slice(k * 8, (k + 1) * 8)
            st8 = bigv[:, ksl, 160:192].bitcast(fp8)
            nc.scalar.activation(
                sq[:, k * 512:(k + 1) * 512].rearrange("p (r f) -> p r f", f=64),
                st8, SQUARE)
            nc.tensor.matmul(
                psum_ssum[:, :],
                lhsT=ones_f8[:, :],
                rhs=st8,
                start=(t == 0 and k == 0),
                stop=(t == last_t and k == kk - 1),
            )
            nc.tensor.matmul(
                psum_ssq[:, :],
                lhsT=ones_bf[:, :],
                rhs=sq[:, k * 512:(k + 1) * 512],
                start=(t == 0 and k == 0),
                stop=(t == last_t and k == kk - 1),
            )

    # ---- device-feat embedding (independent of the loop) ----
    mean_f = ep_pool.tile([64, 1], f32, name="mean_f", tag="mean_f")
    nc.vector.tensor_reduce(mean_f[:, :], dfsT[:, :], axis=AX, op=ADD)
    nc.vector.tensor_scalar_mul(mean_f[:, :], mean_f[:, :], 1.0 / D)
    sqf = ep_pool.tile([64, D], f32, name="sqf", tag="sqf")
    nc.scalar.activation(sqf[:, :], dfsT[:, :], SQUARE)
    qf = ep_pool.tile([64, 1], f32, name="qf", tag="qf")
    nc.vector.tensor_reduce(qf[:, :], sqf[:, :], axis=AX, op=ADD)
    nc.vector.tensor_scalar_mul(qf[:, :], qf[:, :], 1.0 / D)
    varf = ep_pool.tile([64, 1], f32, name="varf", tag="varf")
    nc.vector.tensor_mul(varf[:, :], mean_f[:, :], mean_f[:, :])
    nc.vector.tensor_sub(varf[:, :], qf[:, :], varf[:, :])
    stdf = ep_pool.tile([64, 1], f32, name="stdf", tag="stdf")
    nc.scalar.activation(stdf[:, :], varf[:, :], SQRT)
    nc.vector.tensor_scalar_add(stdf[:, :], stdf[:, :], EPS)
    invf = ep_pool.tile([64, 1], f32, name="invf", tag="invf")
    nc.vector.reciprocal(invf[:, :], stdf[:, :])
    dfsn = ep_pool.tile([64, D], f32, name="dfsn", tag="dfsn")
    nc.vector.tensor_scalar(dfsn[:, :], dfsT[:, :], mean_f[:, :], invf[:, :],
                            op0=SUB, op1=MUL)
    psum_dfe = eppsum_pool.tile([H1, D], f32, name="psum_dfe", tag="ep")
    nc.tensor.matmul(psum_dfe[:, :], lhsT=w1T[:, :], rhs=dfsn[:, :],
                     start=True, stop=True)
    dfeT = ep_pool.tile([H1, D], f32, name="dfeT", tag="dfeT")
    leaky(dfeT[:, :], psum_dfe[:, :], b1[:, :])

    # broadcast mpnn[pred] along the D axis
    repe = ep_pool.tile([H1, D], f32, name="repe", tag="repe")
    nc.scalar.activation(repe[:, :], zeros[:, :], IDENT, bias=mpred[:, :])

    # ---- finish state stats: tree-reduce the [1, 512] rows over the 8
    # column groups, then transpose [1, 64] -> [64, 1] through the PE ----
    srow = ep_pool.tile([1, 512], f32, name="srow", tag="srow")
    qrow = ep_pool.tile([1, 512], f32, name="qrow", tag="qrow")
    nc.vector.tensor_copy(srow[:, :], psum_ssum[:, :])
    nc.vector.tensor_copy(qrow[:, :], psum_ssq[:, :])
    for w in (256, 128, 64):
        nc.vector.tensor_add(srow[:, 0:w], srow[:, 0:w], srow[:, w:2 * w])
        nc.vector.tensor_add(qrow[:, 0:w], qrow[:, 0:w], qrow[:, w:2 * w])
    psum_st = eppsum_pool.tile([64, 2], f32, name="psum_st", tag="ep2")
    nc.tensor.transpose(psum_st[:, 0:1], srow[0:1, 0:64], ident[0:1, 0:1])
    nc.tensor.transpose(psum_st[:, 1:2], qrow[0:1, 0:64], ident[0:1, 0:1])

    # ---- pack + AllReduce ----
    pack = ep_pool.tile([64, 130], f32, name="pack", tag="pack")
    nc.vector.tensor_copy(pack[:, 0:128], psum_dse[:, :])
    nc.vector.tensor_copy(pack[:, 128:130], psum_st[:, :])

    if stage == "loop":
        nc.sync.dma_start(y_dbg[:, :], pack[:, :])
        nc.sync.dma_start(y_out[:], pack[0, 0:64])
        for p in reversed(ctx_pools):
            p.__exit__(None, None, None)
        return

    cc_in = dram_pool.tile([64, 130], f32, name="cc_in", tag="cc_in")
    cc_out = dram_pool.tile([64, 130], f32, name="cc_out", tag="cc_out",
                            addr_space="Shared")
    nc.sync.dma_start(cc_in[:, :], pack[:, :])
    nc.gpsimd.collective_compute(
        "AllReduce",
        ADD,
        replica_groups=[list(range(NCORES))],
        ins=[cc_in[:, :].opt()],
        outs=[cc_out[:, :].opt()],
    )
    red = ep_pool.tile([64, 130], f32, name="red", tag="red")
    nc.sync.dma_start(red[:, :], cc_out[:, :])

    if stage == "pack":
        nc.sync.dma_start(y_dbg[:, :], red[:, :])
        nc.sync.dma_start(y_out[:], red[0, 0:64])
        for p in reversed(ctx_pools):
            p.__exit__(None, None, None)
        return

    # ---- transpose dse [d, h] -> [h, d] (PE) ----
    psum_dseT = eppsum_pool.tile([H1, D], f32, name="psum_dseT", tag="ep")
    nc.tensor.transpose(psum_dseT[:, :], red[0:64, 0:128], ident[0:64, 0:64])

    # ---- state per-feature mean / 1/(std+eps) ----
    mean_s = ep_pool.tile([F, 1], f32, name="mean_s", tag="mean_s")
    nc.vector.tensor_scalar_mul(mean_s[:, :], red[:, 128:129], 1.0 / N)
    ex2_s = ep_pool.tile([F, 1], f32, name="ex2_s", tag="ex2_s")
    nc.vector.tensor_scalar_mul(ex2_s[:, :], red[:, 129:130], 1.0 / N)
    var_s = ep_pool.tile([F, 1], f32, name="var_s", tag="var_s")
    nc.vector.tensor_mul(var_s[:, :], mean_s[:, :], mean_s[:, :])
    nc.vector.tensor_sub(var_s[:, :], ex2_s[:, :], var_s[:, :])
    std_s = ep_pool.tile([F, 1], f32, name="std_s", tag="std_s")
    nc.scalar.activation(std_s[:, :], var_s[:, :], SQRT)
    nc.vector.tensor_scalar_add(std_s[:, :], std_s[:, :], EPS)
    inv_s = ep_pool.tile([F, 1], f32, name="inv_s", tag="inv_s")
    nc.vector.reciprocal(inv_s[:, :], std_s[:, :])

    # rep_latent column: leaky(W2 @ xn + b2), then broadcast over D
    xn = ep_pool.tile([F, 1], f32, name="xn", tag="xn")
    nc.vector.tensor_scalar(xn[:, :], spred[:, :], mean_s[:, :], inv_s[:, :],
                            op0=SUB, op1=MUL)
    psum_repl = eppsum_pool.tile([H1, 1], f32, name="psum_repl", tag="ep2")
    nc.tensor.matmul(psum_repl[:, :], lhsT=w2T[:, :], rhs=xn[:, :],
                     start=True, stop=True)
    repl_c = ep_pool.tile([H1, 1], f32, name="repl_c", tag="repl_c")
    leaky(repl_c[:, :], psum_repl[:, :], b2[:, :])
    repl = ep_pool.tile([H1, D], f32, name="repl", tag="repl")
    nc.scalar.activation(repl[:, :], zeros[:, :], IDENT, bias=repl_c[:, :])

    # dse normalization (over D, free axis)
    dseT = ep_pool.tile([H1, D], f32, name="dseT", tag="dseT")
    nc.vector.tensor_copy(dseT[:, :], psum_dseT[:, :])
    mean_d = ep_pool.tile([H1, 1], f32, name="mean_d", tag="mean_d")
    nc.vector.tensor_reduce(mean_d[:, :], dseT[:, :], axis=AX, op=ADD)
    nc.vector.tensor_scalar_mul(mean_d[:, :], mean_d[:, :], 1.0 / D)
    sqd = ep_pool.tile([H1, D], f32, name="sqd", tag="sqd")
    nc.scalar.activation(sqd[:, :], dseT[:, :], SQUARE)
    qd = ep_pool.tile([H1, 1], f32, name="qd", tag="qd")
    nc.vector.tensor_reduce(qd[:, :], sqd[:, :], axis=AX, op=ADD)
    nc.vector.tensor_scalar_mul(qd[:, :], qd[:, :], 1.0 / D)
    vard = ep_pool.tile([H1, 1], f32, name="vard", tag="vard")
    nc.vector.tensor_mul(vard[:, :], mean_d[:, :], mean_d[:, :])
    nc.vector.tensor_sub(vard[:, :], qd[:, :], vard[:, :])
    stdd = ep_pool.tile([H1, 1], f32, name="stdd", tag="stdd")
    nc.scalar.activation(stdd[:, :], vard[:, :], SQRT)
    nc.vector.tensor_scalar_add(stdd[:, :], stdd[:, :], EPS)
    invd = ep_pool.tile([H1, 1], f32, name="invd", tag="invd")
    nc.vector.reciprocal(invd[:, :], stdd[:, :])
    dsen = ep_pool.tile([H1, D], f32, name="dsen", tag="dsen")
    nc.vector.tensor_scalar(dsen[:, :], dseT[:, :], mean_d[:, :], invd[:, :],
                            op0=SUB, op1=MUL)

    # h.T = leaky(W3 @ concat.T + b3): 4 accumulated chunks over c=512
    psum_h = eppsum_pool.tile([H2, D], f32, name="psum_h", tag="ep")
    chunks = [dfeT[:, :], repl[:, :], repe[:, :], dsen[:, :]]
    for k in range(4):
        nc.tensor.matmul(psum_h[:, :], lhsT=w3Tp[:, k * H2:(k + 1) * H2],
                         rhs=chunks[k], start=(k == 0), stop=(k == 3))
    hT = ep_pool.tile([H2, D], f32, name="hT", tag="hT")
    leaky(hT[:, :], psum_h[:, :], b3[:, :])

    # output[d] = sum_j hT[j, d] * W4[0, j] + b4
    psum_o = eppsum_pool.tile([D, 1], f32, name="psum_o", tag="ep2")
    nc.tensor.matmul(psum_o[:, :], lhsT=hT[:, :], rhs=w4T[:, :],
                     start=True, stop=True)
    out_sb = ep_pool.tile([D, 1], f32, name="out_sb", tag="out_sb")
    nc.scalar.activation(out_sb[:, :], psum_o[:, :], IDENT, bias=b4[:, :])
    nc.sync.dma_start(y_out[:], out_sb[:, 0])

    for p in reversed(ctx_pools):
        p.__exit__(None, None, None)


_compiled = None


def _get_compiled():
    global _compiled
    if _compiled is None:
        _compiled = build_program()
    return _compiled


def make_in_maps(inputs):
    bf = ml_dtypes.bfloat16
    state = np.asarray(inputs["state"], dtype=np.float32)
    dfs = np.asarray(inputs["device_feat_state"], dtype=np.float32)
    mpnn = np.asarray(inputs["mpnn_forward"], dtype=np.float32)
    W1 = np.asarray(inputs["W1"], dtype=np.float32)
    b1 = np.asarray(inputs["b1"], dtype=np.float32)
    W2 = np.asarray(inputs["W2"], dtype=np.float32)
    b2 = np.asarray(inputs["b2"], dtype=np.float32)
    W3 = np.asarray(inputs["W3"], dtype=np.float32)
    b3 = np.asarray(inputs["b3"], dtype=np.float32)
    W4 = np.asarray(inputs["W4"], dtype=np.float32)
    b4 = np.asarray(inputs["b4"], dtype=np.float32)
    mask = np.asarray(inputs["device_assign_state"])
    assert mask.dtype == np.int32
    pred = int(np.asarray(inputs["pred_node"]))

    f8 = ml_dtypes.float8_e4m3fn
    # mask 0/1 and 1.0 are exact in fp8 e4m3; state stats tolerate fp8
    Xb = np.empty((N, 384), dtype=np.uint8)
    np.copyto(Xb[:, 0:64].view(f8), mask.T, casting="unsafe")
    Xb[:, 64:320] = mpnn.astype(bf).view(np.uint8)
    Xb[:, 320:384] = state.astype(f8).view(np.uint8)
    X16 = Xb.view(np.uint16)

    w3Tp = np.ascontiguousarray(
        W3.T.reshape(4, H1, H2).transpose(1, 0, 2).reshape(H1, 4 * H2))
    common = {
        "x_dfsT": np.ascontiguousarray(np.pad(dfs.T, ((0, 64 - DF), (0, 0)))),
        "x_w1T": np.ascontiguousarray(np.pad(W1.T, ((0, 64 - DF), (0, 0)))),
        "x_b1": np.ascontiguousarray(b1.reshape(H1, 1)),
        "x_w2T": np.ascontiguousarray(W2.T),
        "x_b2": np.ascontiguousarray(b2.reshape(H1, 1)),
        "x_w3Tp": w3Tp,
        "x_b3": np.ascontiguousarray(b3.reshape(H2, 1)),
        "x_w4T": np.ascontiguousarray(W4.T),
        "x_b4": np.ascontiguousarray(np.broadcast_to(b4.reshape(1, 1), (D, 1))),
        "x_spred": np.ascontiguousarray(state[pred].reshape(F, 1)),
        "x_mpred": np.ascontiguousarray(mpnn[pred].reshape(H1, 1)),
    }
    in_maps = []
    for c in range(NCORES):
        in_maps.append({
            **common,
            "x_raw": X16[c * NSH:(c + 1) * NSH],
        })
    return in_maps


def kernel(**inputs) -> np.ndarray:
    nc = _get_compiled()
    in_maps = make_in_maps(inputs)
    res = run_bass_kernel_spmd(nc, in_maps, core_ids=list(range(NCORES)))
    return np.asarray(res.results[0]["y_out"], dtype=np.float32)


# revision 19
# speedup vs baseline: 1.0675x; 1.0675x over previous
"""Trainium2 Bass kernel for nn_Device_Policy (segment_reduce).

Strategy: shard the node axis N across 8 NeuronCores.  Host marshals one
interleaved bf16 tensor X[N, 256] per row:
    [ maskT(64) | mpnn(128) | state(64) ]
(mask 0/1 is exact in bf16; mpnn/state bf16 rounding ~0.4% rel, well under
the 2e-2 gate).  Rows are self-contained, so nodes can be laid out in any
(partition, slot) order: each SBUF tile is loaded as one fully-contiguous
16 KiB run per partition (128 descriptors per 2 MiB tile -> DMA runs at
full HBM bandwidth).  Per 128-node chunk the PE does one bf16 matmul
    lhsT = maskT  (K=128 nodes, M=64)   rhs = mpnn  (N=128)
accumulating dse[d, h] into a single PSUM [64, 128] over all 256 chunks.
State column sums / sums-of-squares accumulate on the otherwise-idle DVE
(f32 accumulators, scalar-engine Square), finished with two tiny
ones-matmuls.  The [64, 130] partial is AllReduce'd across the 8 cores and
the small replicated MLP head runs after (device-feat embedding
precomputed before the loop).
"""

import sys

if "/opt/trn_rl_repo" not in sys.path:
    sys.path.insert(0, "/opt/trn_rl_repo")

import ml_dtypes
import numpy as np

import concourse.bacc as bacc
import concourse.bass as bass
import concourse.mybir as mybir
import concourse.tile as tile
from concourse import masks
from concourse.bass_utils import run_bass_kernel_spmd

NCORES = 8
N = 262144
F = 64
D = 64
DF = 32
H1 = 128
H2 = 64
NSH = N // NCORES          # nodes per core = 32768
TN = 4096                  # nodes per tile
NT = NSH // TN             # 8 tiles per core
R = TN // 128              # 32 row-slots per partition = chunks per tile
XW = 256                   # shipped row width (bf16): mask|mpnn|state
EPS = 1e-6
SLOPE = 0.1
# Fused Lrelu mis-computes on HW (alpha is not honored) and CoreSim does not
# implement it either -- keep the mul+max decomposition.
USE_LRELU = False

f32 = mybir.dt.float32
bf16 = mybir.dt.bfloat16
fp8 = mybir.dt.float8e4
ADD = mybir.AluOpType.add
MUL = mybir.AluOpType.mult
SUB = mybir.AluOpType.subtract
AX = mybir.AxisListType.X
LRELU = mybir.ActivationFunctionType.Lrelu
IDENT = mybir.ActivationFunctionType.Identity
SQUARE = mybir.ActivationFunctionType.Square
SQRT = mybir.ActivationFunctionType.Sqrt


def build_program(stage="full"):
    nc = bacc.Bacc(
        "TRN2",
        target_bir_lowering=False,
        debug=False,
        enable_asserts=False,
        num_devices=NCORES,
    )

    x_mask8 = nc.dram_tensor("x_mask8", [NSH, 64], fp8, kind="ExternalInput")
    x_mpnn = nc.dram_tensor("x_mpnn", [NSH, H1], bf16, kind="ExternalInput")
    x_state8 = nc.dram_tensor("x_state8", [NSH, 64], fp8, kind="ExternalInput")
    x_dfsT = nc.dram_tensor("x_dfsT", [64, D], f32, kind="ExternalInput")
    x_w1T = nc.dram_tensor("x_w1T", [64, H1], f32, kind="ExternalInput")
    x_b1 = nc.dram_tensor("x_b1", [H1, 1], f32, kind="ExternalInput")
    x_w2T = nc.dram_tensor("x_w2T", [F, H1], f32, kind="ExternalInput")
    x_b2 = nc.dram_tensor("x_b2", [H1, 1], f32, kind="ExternalInput")
    x_w3Tp = nc.dram_tensor("x_w3Tp", [H1, 4 * H2], f32, kind="ExternalInput")
    x_b3 = nc.dram_tensor("x_b3", [H2, 1], f32, kind="ExternalInput")
    x_w4T = nc.dram_tensor("x_w4T", [H2, 1], f32, kind="ExternalInput")
    x_b4 = nc.dram_tensor("x_b4", [D, 1], f32, kind="ExternalInput")
    x_spred = nc.dram_tensor("x_spred", [F, 1], f32, kind="ExternalInput")
    x_mpred = nc.dram_tensor("x_mpred", [H1, 1], f32, kind="ExternalInput")
    y_out = nc.dram_tensor("y_out", [D], f32, kind="ExternalOutput")
    y_dbg = None
    if stage != "full":
        y_dbg = nc.dram_tensor("y_dbg", [64, 130], f32, kind="ExternalOutput")

    with tile.TileContext(nc) as tc:
        emit(nc, tc, x_mask8, x_mpnn, x_state8, x_dfsT, x_w1T, x_b1, x_w2T, x_b2, x_w3Tp, x_b3,
             x_w4T, x_b4, x_spred, x_mpred, y_out, stage=stage, y_dbg=y_dbg)

    nc.compile()
    return nc


def emit(nc, tc, x_mask8, x_mpnn, x_state8, x_dfsT, x_w1T, x_b1, x_w2T, x_b2, x_w3Tp, x_b3,
         x_w4T, x_b4, x_spred, x_mpred, y_out, stage="full", y_dbg=None):
    ctx_pools = []

    def pool(name, bufs, space="SBUF"):
        p = tc.tile_pool(name=name, bufs=bufs, space=space)
        ctx_pools.append(p)
        return p.__enter__()

    cpool = pool("const", 1)
    big_pool = pool("big", 4)
    sq_pool = pool("sq", 4)
    ep_pool = pool("ep", 1)
    dsepsum_pool = pool("dsepsum", 1, space="PSUM")
    eppsum_pool = pool("eppsum", 2, space="PSUM")
    dram_pool = pool("dram", 1, space="DRAM")

    def leaky(out_t, psum_in, bias):
        if USE_LRELU:
            nc.scalar.activation(out_t, psum_in, LRELU, bias=bias, alpha=SLOPE)
        else:
            nc.scalar.activation(out_t, psum_in, IDENT, bias=bias)
            sc = ep_pool.tile(list(out_t.shape), f32, name="lk", tag="lk")
            nc.vector.tensor_scalar_mul(sc[:, :], out_t, SLOPE)
            nc.vector.tensor_max(out_t, out_t, sc[:, :])

    # ---- constants ----
    ident = cpool.tile([128, 128], f32, name="ident")
    masks.make_identity(nc, ident[:, :])
    zeros = cpool.tile([128, D], f32, name="zeros")
    nc.vector.memset(zeros[:, :], 0.0)

    # ---- warm up the CC engine with a tiny dummy collective: the first
    # collective trigger pays ~10us of CC launch; doing it early (overlapped
    # with the loop) makes the real AllReduce start promptly ----
    dumb_in = dram_pool.tile([1, 1], f32, name="dumb_in", tag="dumb_in")
    dumb_out = dram_pool.tile([1, 1], f32, name="dumb_out", tag="dumb_out",
                              addr_space="Shared")
    nc.scalar.dma_start(dumb_in[:, :], zeros[0:1, 0:1])
    nc.gpsimd.collective_compute(
        "AllReduce",
        ADD,
        replica_groups=[list(range(NCORES))],
        ins=[dumb_in[:, :].opt()],
        outs=[dumb_out[:, :].opt()],
    )

    ones_bf = cpool.tile([128, 1], bf16, name="ones_bf")
    nc.vector.memset(ones_bf[:, :], 1.0)
    ones_f8 = cpool.tile([128, 1], fp8, name="ones_f8")
    nc.vector.memset(ones_f8[:, :], 1.0)

    dfsT = cpool.tile([64, D], f32, name="dfsT")
    nc.scalar.dma_start(dfsT[:, :], x_dfsT[:, :])
    w1T = cpool.tile([64, H1], f32, name="w1T")
    nc.scalar.dma_start(w1T[:, :], x_w1T[:, :])
    b1 = cpool.tile([H1, 1], f32, name="b1")
    nc.scalar.dma_start(b1[:, :], x_b1[:, :])
    w2T = cpool.tile([F, H1], f32, name="w2T")
    nc.scalar.dma_start(w2T[:, :], x_w2T[:, :])
    b2 = cpool.tile([H1, 1], f32, name="b2")
    nc.scalar.dma_start(b2[:, :], x_b2[:, :])
    w3Tp = cpool.tile([H1, 4 * H2], f32, name="w3Tp")
    nc.scalar.dma_start(w3Tp[:, :], x_w3Tp[:, :])
    b3 = cpool.tile([H2, 1], f32, name="b3")
    nc.scalar.dma_start(b3[:, :], x_b3[:, :])
    w4T = cpool.tile([H2, 1], f32, name="w4T")
    nc.scalar.dma_start(w4T[:, :], x_w4T[:, :])
    b4 = cpool.tile([D, 1], f32, name="b4")
    nc.scalar.dma_start(b4[:, :], x_b4[:, :])
    spred = cpool.tile([F, 1], f32, name="spred")
    nc.scalar.dma_start(spred[:, :], x_spred[:, :])
    mpred = cpool.tile([H1, 1], f32, name="mpred")
    nc.scalar.dma_start(mpred[:, :], x_mpred[:, :])

    # ---- main loop ----
    # small first tiles (earlier first matmul) and small last tiles
    # (shorter drain); 384 B per node row across three tensors
    tiles = []
    off = 0
    for n in [1024, 1024, 2048, 4096, 4096, 4096, 4096, 4096, 4096, 2048, 1024, 1024]:
        tiles.append((off, n))
        off += n
    assert off == NSH
    last_t = len(tiles) - 1
    psum_dse = dsepsum_pool.tile([D, H1], f32, name="psum_dse", tag="psum_dse")
    psum_ssum = dsepsum_pool.tile([1, 512], f32, name="psum_ssum", tag="pss")
    psum_ssq = dsepsum_pool.tile([1, 512], f32, name="psum_ssq", tag="psq")
    for t, (off, n) in enumerate(tiles):
        rr = n // 128
        kk = rr // 8
        mk = big_pool.tile([128, R * 64], fp8, name="mk", tag="mk")
        mp = big_pool.tile([128, R * H1], bf16, name="mp", tag="mp")
        st = big_pool.tile([128, R * 64], fp8, name="st", tag="st")
        mkv = mk[:, :].rearrange("p (r f) -> p r f", f=64)
        mpv = mp[:, :].rearrange("p (r f) -> p r f", f=H1)
        # node = off + rr*p + r: each partition is one contiguous run
        nc.sync.dma_start(
            mk[:, 0:rr * 64],
            x_mask8[off:off + n, :].rearrange("(p r) f -> p (r f)", r=rr))
        nc.sync.dma_start(
            mp[:, 0:rr * H1],
            x_mpnn[off:off + n, :].rearrange("(p r) f -> p (r f)", r=rr))
        nc.scalar.dma_start(
            st[:, 0:rr * 64],
            x_state8[off:off + n, :].rearrange("(p r) f -> p (r f)", r=rr))
        sq = sq_pool.tile([128, R * 64], bf16, name="sq", tag="sq")
        for r in range(rr):
            nc.tensor.matmul(
                psum_dse[:, :],
                lhsT=mkv[:, r, :],
                rhs=mpv[:, r, :],
                start=(t == 0 and r == 0),
                stop=(t == last_t and r == rr - 1),
            )
        # state column sums / sums-of-squares ride the PE too: ones-weight
        # matmuls reduce over the partition (node) axis, 512 columns a pop
        for k in range(kk):
            ksl = slice(k * 512, (k + 1) * 512)
            nc.scalar.activation(sq[:, ksl], st[:, ksl], SQUARE)
            nc.tensor.matmul(
                psum_ssum[:, :],
                lhsT=ones_f8[:, :],
                rhs=st[:, ksl],
                start=(t == 0 and k == 0),
                stop=(t == last_t and k == kk - 1),
            )
            nc.tensor.matmul(
                psum_ssq[:, :],
                lhsT=ones_bf[:, :],
                rhs=sq[:, ksl],
                start=(t == 0 and k == 0),
                stop=(t == last_t and k == kk - 1),
            )

    # ---- device-feat embedding (independent of the loop) ----
    mean_f = ep_pool.tile([64, 1], f32, name="mean_f", tag="mean_f")
    nc.vector.tensor_reduce(mean_f[:, :], dfsT[:, :], axis=AX, op=ADD)
    nc.vector.tensor_scalar_mul(mean_f[:, :], mean_f[:, :], 1.0 / D)
    sqf = ep_pool.tile([64, D], f32, name="sqf", tag="sqf")
    nc.scalar.activation(sqf[:, :], dfsT[:, :], SQUARE)
    qf = ep_pool.tile([64, 1], f32, name="qf", tag="qf")
    nc.vector.tensor_reduce(qf[:, :], sqf[:, :], axis=AX, op=ADD)
    nc.vector.tensor_scalar_mul(qf[:, :], qf[:, :], 1.0 / D)
    varf = ep_pool.tile([64, 1], f32, name="varf", tag="varf")
    nc.vector.tensor_mul(varf[:, :], mean_f[:, :], mean_f[:, :])
    nc.vector.tensor_sub(varf[:, :], qf[:, :], varf[:, :])
    stdf = ep_pool.tile([64, 1], f32, name="stdf", tag="stdf")
    nc.scalar.activation(stdf[:, :], varf[:, :], SQRT)
    nc.vector.tensor_scalar_add(stdf[:, :], stdf[:, :], EPS)
    invf = ep_pool.tile([64, 1], f32, name="invf", tag="invf")
    nc.vector.reciprocal(invf[:, :], stdf[:, :])
    dfsn = ep_pool.tile([64, D], f32, name="dfsn", tag="dfsn")
    nc.vector.tensor_scalar(dfsn[:, :], dfsT[:, :], mean_f[:, :], invf[:, :],
                            op0=SUB, op1=MUL)
    psum_dfe = eppsum_pool.tile([H1, D], f32, name="psum_dfe", tag="ep")
    nc.tensor.matmul(psum_dfe[:, :], lhsT=w1T[:, :], rhs=dfsn[:, :],
                     start=True, stop=True)
    dfeT = ep_pool.tile([H1, D], f32, name="dfeT", tag="dfeT")
    leaky(dfeT[:, :], psum_dfe[:, :], b1[:, :])

    # broadcast mpnn[pred] along the D axis
    repe = ep_pool.tile([H1, D], f32, name="repe", tag="repe")
    nc.scalar.activation(repe[:, :], zeros[:, :], IDENT, bias=mpred[:, :])

    # ---- finish state stats: tree-reduce the [1, 512] rows over the 8
    # column groups, then transpose [1, 64] -> [64, 1] through the PE ----
    srow = ep_pool.tile([1, 512], f32, name="srow", tag="srow")
    qrow = ep_pool.tile([1, 512], f32, name="qrow", tag="qrow")
    nc.vector.tensor_copy(srow[:, :], psum_ssum[:, :])
    nc.vector.tensor_copy(qrow[:, :], psum_ssq[:, :])
    for w in (256, 128, 64):
        nc.vector.tensor_add(srow[:, 0:w], srow[:, 0:w], srow[:, w:2 * w])
        nc.vector.tensor_add(qrow[:, 0:w], qrow[:, 0:w], qrow[:, w:2 * w])
    psum_st = eppsum_pool.tile([64, 2], f32, name="psum_st", tag="ep2")
    nc.tensor.transpose(psum_st[:, 0:1], srow[0:1, 0:64], ident[0:1, 0:1])
    nc.tensor.transpose(psum_st[:, 1:2], qrow[0:1, 0:64], ident[0:1, 0:1])

    # ---- pack + AllReduce ----
    pack = ep_pool.tile([64, 130], f32, name="pack", tag="pack")
    nc.vector.tensor_copy(pack[:, 0:128], psum_dse[:, :])
    nc.vector.tensor_copy(pack[:, 128:130], psum_st[:, :])

    if stage == "loop":
        nc.sync.dma_start(y_dbg[:, :], pack[:, :])
        nc.sync.dma_start(y_out[:], pack[0, 0:64])
        for p in reversed(ctx_pools):
            p.__exit__(None, None, None)
        return

    cc_in = dram_pool.tile([64, 130], f32, name="cc_in", tag="cc_in")
    cc_out = dram_pool.tile([64, 130], f32, name="cc_out", tag="cc_out",
                            addr_space="Shared")
    nc.sync.dma_start(cc_in[:, :], pack[:, :])
    nc.gpsimd.collective_compute(
        "AllReduce",
        ADD,
        replica_groups=[list(range(NCORES))],
        ins=[cc_in[:, :].opt()],
        outs=[cc_out[:, :].opt()],
    )
    red = ep_pool.tile([64, 130], f32, name="red", tag="red")
    nc.sync.dma_start(red[:, :], cc_out[:, :])

    if stage == "pack":
        nc.sync.dma_start(y_dbg[:, :], red[:, :])
        nc.sync.dma_start(y_out[:], red[0, 0:64])
        for p in reversed(ctx_pools):
            p.__exit__(None, None, None)
        return

    # ---- transpose dse [d, h] -> [h, d] (PE) ----
    psum_dseT = eppsum_pool.tile([H1, D], f32, name="psum_dseT", tag="ep")
    nc.tensor.transpose(psum_dseT[:, :], red[0:64, 0:128], ident[0:64, 0:64])

    # ---- state per-feature mean / 1/(std+eps) ----
    mean_s = ep_pool.tile([F, 1], f32, name="mean_s", tag="mean_s")
    nc.vector.tensor_scalar_mul(mean_s[:, :], red[:, 128:129], 1.0 / N)
    ex2_s = ep_pool.tile([F, 1], f32, name="ex2_s", tag="ex2_s")
    nc.vector.tensor_scalar_mul(ex2_s[:, :], red[:, 129:130], 1.0 / N)
    var_s = ep_pool.tile([F, 1], f32, name="var_s", tag="var_s")
    nc.vector.tensor_mul(var_s[:, :], mean_s[:, :], mean_s[:, :])
    nc.vector.tensor_sub(var_s[:, :], ex2_s[:, :], var_s[:, :])
    std_s = ep_pool.tile([F, 1], f32, name="std_s", tag="std_s")
    nc.scalar.activation(std_s[:, :], var_s[:, :], SQRT)
    nc.vector.tensor_scalar_add(std_s[:, :], std_s[:, :], EPS)
    inv_s = ep_pool.tile([F, 1], f32, name="inv_s", tag="inv_s")
    nc.vector.reciprocal(inv_s[:, :], std_s[:, :])

    # rep_latent column: leaky(W2 @ xn + b2), then broadcast over D
    xn = ep_pool.tile([F, 1], f32, name="xn", tag="xn")
    nc.vector.tensor_scalar(xn[:, :], spred[:, :], mean_s[:, :], inv_s[:, :],
                            op0=SUB, op1=MUL)
    psum_repl = eppsum_pool.tile([H1, 1], f32, name="psum_repl", tag="ep2")
    nc.tensor.matmul(psum_repl[:, :], lhsT=w2T[:, :], rhs=xn[:, :],
                     start=True, stop=True)
    repl_c = ep_pool.tile([H1, 1], f32, name="repl_c", tag="repl_c")
    leaky(repl_c[:, :], psum_repl[:, :], b2[:, :])
    repl = ep_pool.tile([H1, D], f32, name="repl", tag="repl")
    nc.scalar.activation(repl[:, :], zeros[:, :], IDENT, bias=repl_c[:, :])

    # dse normalization (over D, free axis)
    dseT = ep_pool.tile([H1, D], f32, name="dseT", tag="dseT")
    nc.vector.tensor_copy(dseT[:, :], psum_dseT[:, :])
    mean_d = ep_pool.tile([H1, 1], f32, name="mean_d", tag="mean_d")
    nc.vector.tensor_reduce(mean_d[:, :], dseT[:, :], axis=AX, op=ADD)
    nc.vector.tensor_scalar_mul(mean_d[:, :], mean_d[:, :], 1.0 / D)
    sqd = ep_pool.tile([H1, D], f32, name="sqd", tag="sqd")
    nc.scalar.activation(sqd[:, :], dseT[:, :], SQUARE)
    qd = ep_pool.tile([H1, 1], f32, name="qd", tag="qd")
    nc.vector.tensor_reduce(qd[:, :], sqd[:, :], axis=AX, op=ADD)
    nc.vector.tensor_scalar_mul(qd[:, :], qd[:, :], 1.0 / D)
    vard = ep_pool.tile([H1, 1], f32, name="vard", tag="vard")
    nc.vector.tensor_mul(vard[:, :], mean_d[:, :], mean_d[:, :])
    nc.vector.tensor_sub(vard[:, :], qd[:, :], vard[:, :])
    stdd = ep_pool.tile([H1, 1], f32, name="stdd", tag="stdd")
    nc.scalar.activation(stdd[:, :], vard[:, :], SQRT)
    nc.vector.tensor_scalar_add(stdd[:, :], stdd[:, :], EPS)
    invd = ep_pool.tile([H1, 1], f32, name="invd", tag="invd")
    nc.vector.reciprocal(invd[:, :], stdd[:, :])
    dsen = ep_pool.tile([H1, D], f32, name="dsen", tag="dsen")
    nc.vector.tensor_scalar(dsen[:, :], dseT[:, :], mean_d[:, :], invd[:, :],
                            op0=SUB, op1=MUL)

    # h.T = leaky(W3 @ concat.T + b3): 4 accumulated chunks over c=512
    psum_h = eppsum_pool.tile([H2, D], f32, name="psum_h", tag="ep")
    chunks = [dfeT[:, :], repl[:, :], repe[:, :], dsen[:, :]]
    for k in range(4):
        nc.tensor.matmul(psum_h[:, :], lhsT=w3Tp[:, k * H2:(k + 1) * H2],
                         rhs=chunks[k], start=(k == 0), stop=(k == 3))
    hT = ep_pool.tile([H2, D], f32, name="hT", tag="hT")
    leaky(hT[:, :], psum_h[:, :], b3[:, :])

    # output[d] = sum_j hT[j, d] * W4[0, j] + b4
    psum_o = eppsum_pool.tile([D, 1], f32, name="psum_o", tag="ep2")
    nc.tensor.matmul(psum_o[:, :], lhsT=hT[:, :], rhs=w4T[:, :],
                     start=True, stop=True)
    out_sb = ep_pool.tile([D, 1], f32, name="out_sb", tag="out_sb")
    nc.scalar.activation(out_sb[:, :], psum_o[:, :], IDENT, bias=b4[:, :])
    nc.sync.dma_start(y_out[:], out_sb[:, 0])

    for p in reversed(ctx_pools):
        p.__exit__(None, None, None)


_compiled = None


def _get_compiled():
    global _compiled
    if _compiled is None:
        _compiled = build_program()
    return _compiled


def make_in_maps(inputs):
    bf = ml_dtypes.bfloat16
    state = np.asarray(inputs["state"], dtype=np.float32)
    dfs = np.asarray(inputs["device_feat_state"], dtype=np.float32)
    mpnn = np.asarray(inputs["mpnn_forward"], dtype=np.float32)
    W1 = np.asarray(inputs["W1"], dtype=np.float32)
    b1 = np.asarray(inputs["b1"], dtype=np.float32)
    W2 = np.asarray(inputs["W2"], dtype=np.float32)
    b2 = np.asarray(inputs["b2"], dtype=np.float32)
    W3 = np.asarray(inputs["W3"], dtype=np.float32)
    b3 = np.asarray(inputs["b3"], dtype=np.float32)
    W4 = np.asarray(inputs["W4"], dtype=np.float32)
    b4 = np.asarray(inputs["b4"], dtype=np.float32)
    mask = np.asarray(inputs["device_assign_state"])
    assert mask.dtype == np.int32
    pred = int(np.asarray(inputs["pred_node"]))

    f8 = ml_dtypes.float8_e4m3fn
    # mask 0/1 and 1.0 are exact in fp8 e4m3; state stats tolerate fp8
    Xmask = np.empty((N, 64), dtype=f8)
    np.copyto(Xmask, mask.T, casting="unsafe")
    Xmpnn = mpnn.astype(bf)
    Xstate = state.astype(f8)

    w3Tp = np.ascontiguousarray(
        W3.T.reshape(4, H1, H2).transpose(1, 0, 2).reshape(H1, 4 * H2))
    common = {
        "x_dfsT": np.ascontiguousarray(np.pad(dfs.T, ((0, 64 - DF), (0, 0)))),
        "x_w1T": np.ascontiguousarray(np.pad(W1.T, ((0, 64 - DF), (0, 0)))),
        "x_b1": np.ascontiguousarray(b1.reshape(H1, 1)),
        "x_w2T": np.ascontiguousarray(W2.T),
        "x_b2": np.ascontiguousarray(b2.reshape(H1, 1)),
        "x_w3Tp": w3Tp,
        "x_b3": np.ascontiguousarray(b3.reshape(H2, 1)),
        "x_w4T": np.ascontiguousarray(W4.T),
        "x_b4": np.ascontiguousarray(np.broadcast_to(b4.reshape(1, 1), (D, 1))),
        "x_spred": np.ascontiguousarray(state[pred].reshape(F, 1)),
        "x_mpred": np.ascontiguousarray(mpnn[pred].reshape(H1, 1)),
    }
    in_maps = []
    for c in range(NCORES):
        sl = slice(c * NSH, (c + 1) * NSH)
        in_maps.append({
            **common,
            "x_mask8": Xmask[sl],
            "x_mpnn": Xmpnn[sl],
            "x_state8": Xstate[sl],
        })
    return in_maps


def kernel(**inputs) -> np.ndarray:
    nc = _get_compiled()
    in_maps = make_in_maps(inputs)
    res = run_bass_kernel_spmd(nc, in_maps, core_ids=list(range(NCORES)))
    return np.asarray(res.results[0]["y_out"], dtype=np.float32)
